# revision 8
# baseline (speedup 1.0000x reference)
"""
Trainium2 Bass kernel for nn_DisjointDecoderAE.

  encoder (shared MLP):  x[B,U] -> relu x3 -> z[B,L]
  decoder (U disjoint MLPs, stacked weights): z -> relu(64) -> relu(64) -> relu(64) -> scalar

Sharding: encoder replicated on every core (it is tiny); decoder expert-parallel
over the unit axis U (64 units per core x 8 cores).

PE cost on TRN2 is output_columns x 0.417ns regardless of M/K utilization, so
the kernel packs TWO units per matmul: block-diagonal [128,128] stationary
tiles for L2/L3 (K=2x64, M=2x64), [64,128] for L1 (K=2x32), and [128,32]
32-col-aligned slots for L4 (4 pairs share one PSUM accumulator, DMA'd
straight from PSUM to DRAM).  This halves L2/L3 matmul count vs one-unit
quadrant packing: 540 matmuls x 512 cols ~= 115us PE floor.

PSUM->SBUF drains (relu+bias) are [128,512] single-bank ops balanced across
VectorE, ScalarE and GpSimd (Pool) by measured per-op cost; the decoder is a
1-slot software pipeline (L1(j) | L2(j-1) | L3(j-2) | L4(j-3)) so the PE
never waits on a drain.

Self-contained: shapes/sharding hardcoded; host packs weights, device
computes, host re-assembles (final transpose + bd4 bias on host).
"""

import os
import sys

sys.path.insert(0, "/opt/trn_rl_repo")

import numpy as np
import ml_dtypes

import concourse.bass as bass
import concourse.mybir as mybir
import concourse.tile as tile
from concourse import bacc
from concourse.bass_utils import run_bass_kernel_spmd

B, U, L, H = 2048, 512, 32, 64
NCORES = 8
UC = U // NCORES          # 64 units per core
NP = UC // 2              # 32 unit-pairs per core
CH = 512                  # one fp32 PSUM bank of batch
CP = 1024                 # per-j batch span (2 chunks)
NCP = B // CP             # 2 chunk-pairs
NCHUNK = B // CH          # 4 chunks
KT = U // 128             # 4 k-tiles for encoder layer 1
NSLOT = NCP * NP          # 64 pipeline slots

BF16 = mybir.dt.bfloat16
FP32 = mybir.dt.float32
NPBF = ml_dtypes.bfloat16

LAST_EXEC_NS = None
LAST_RESULTS = None
_PROG = None


def _pack_shared(x, We1, be1, We2, be2, We3, be3, We4, be4):
    xT = np.ascontiguousarray(x.T).astype(NPBF)              # [U, B]
    xt = np.ascontiguousarray(xT.reshape(KT, 128, B))        # k-tiles
    wenc = np.zeros((128, 512), np.float32)
    wenc[:, 0:KT * H] = We1.reshape(KT, 128, H).transpose(1, 0, 2).reshape(
        128, KT * H)
    wenc[0:H, 256:320] = We2
    wenc[0:H, 320:384] = We3
    wenc[0:H, 384:448] = np.tile(We4, (1, 2))                # 2 z copies
    benc = np.zeros((128, 4), np.float32)
    benc[0:H, 0] = be1
    benc[0:H, 1] = be2
    benc[0:H, 2] = be3
    benc[0:H, 3] = np.tile(be4, 2)
    return dict(xt=xt, wenc=wenc.astype(NPBF), benc=benc)


def _pack_core(c, Wd1, bd1, Wd2, bd2, Wd3, bd3, Wd4):
    u0 = c * UC
    w1 = Wd1[u0:u0 + UC]          # [UC, L, H]
    b1 = bd1[u0:u0 + UC]
    w2 = Wd2[u0:u0 + UC]          # [UC, H, H]
    b2 = bd2[u0:u0 + UC]
    w3 = Wd3[u0:u0 + UC]
    b3 = bd3[u0:u0 + UC]
    w4 = Wd4[u0:u0 + UC]          # [UC, H]

    # L1: per pair p, [64,128] lhsT: rows 0:32 unit 2p -> cols 0:64,
    # rows 32:64 unit 2p+1 -> cols 64:128 (rhs = two stacked z copies).
    wd1p = np.zeros((64, NP * 128), np.float32)
    # L2/L3: per pair, [128,128] block-diagonal.
    wd2p = np.zeros((128, NP * 128), np.float32)
    wd3p = np.zeros((128, NP * 128), np.float32)
    # L4: per pair, [128,32] slot: col 0 rows 0:64 = w4[2p], col 1 rows
    # 64:128 = w4[2p+1]; loaded at PE column 32*(p%4).
    wd4p = np.zeros((128, NP * 32), np.float32)
    for p in range(NP):
        wd1p[0:32, p * 128 + 0:p * 128 + 64] = w1[2 * p]
        wd1p[32:64, p * 128 + 64:p * 128 + 128] = w1[2 * p + 1]
        wd2p[0:64, p * 128 + 0:p * 128 + 64] = w2[2 * p]
        wd2p[64:128, p * 128 + 64:p * 128 + 128] = w2[2 * p + 1]
        wd3p[0:64, p * 128 + 0:p * 128 + 64] = w3[2 * p]
        wd3p[64:128, p * 128 + 64:p * 128 + 128] = w3[2 * p + 1]
        wd4p[0:64, 32 * p] = w4[2 * p]
        wd4p[64:128, 32 * p + 1] = w4[2 * p + 1]

    bdec = np.zeros((128, 96), np.float32)
    for p in range(NP):
        bdec[0:64, p] = b1[2 * p]
        bdec[64:128, p] = b1[2 * p + 1]
        bdec[0:64, 32 + p] = b2[2 * p]
        bdec[64:128, 32 + p] = b2[2 * p + 1]
        bdec[0:64, 64 + p] = b3[2 * p]
        bdec[64:128, 64 + p] = b3[2 * p + 1]
    return dict(wd1=wd1p.astype(NPBF), wd2=wd2p.astype(NPBF),
                wd3=wd3p.astype(NPBF), wd4=wd4p.astype(NPBF), bdec=bdec)


class _Drain:
    """Weighted VectorE/ScalarE alternation for PSUM->SBUF drains using
    HW-measured per-op costs.  GPSIMD cannot read PSUM and DMA cannot read
    PSUM (BIR verifier), so DVE+ACT are the only drain engines on TRN2."""

    def __init__(self, nc):
        self.nc = nc
        self.t_dve = 0.0
        self.t_act = 0.0

    def __call__(self, out, psum, bias=None, relu=False):
        fd = 1
        for step, cnt in psum.ap[1:]:
            fd *= cnt
        dve_ns = (120.0 + fd) / 0.96 + 88.0
        act_ns = (172.0 + fd) / 1.2 + 117.0
        nc = self.nc
        if self.t_dve + dve_ns <= self.t_act + act_ns:
            self.t_dve += dve_ns
            if relu:
                nc.vector.tensor_scalar(out, psum, bias, 0.0,
                                        op0=mybir.AluOpType.add,
                                        op1=mybir.AluOpType.max)
            elif bias is not None:
                nc.vector.tensor_scalar(out, psum, bias, None,
                                        op0=mybir.AluOpType.add)
            else:
                nc.vector.tensor_copy(out, psum)
        else:
            self.t_act += act_ns
            if relu:
                nc.scalar.activation(out, psum, mybir.ActivationFunctionType.Relu,
                                     bias=bias)
            elif bias is not None:
                nc.scalar.activation(out, psum,
                                     mybir.ActivationFunctionType.Identity,
                                     bias=bias)
            else:
                nc.scalar.copy(out, psum)


def _build_program():
    nc = bacc.Bacc("TRN2", target_bir_lowering=False, debug=False)

    def din(name, shape, dtype):
        return nc.dram_tensor(name, list(shape), dtype, kind="ExternalInput").ap()

    xt_d = din("xt", (KT, 128, B), BF16)
    wenc_d = din("wenc", (128, 512), BF16)
    benc_d = din("benc", (128, 4), FP32)
    wd1_d = din("wd1", (64, NP * 128), BF16)
    wd2_d = din("wd2", (128, NP * 128), BF16)
    wd3_d = din("wd3", (128, NP * 128), BF16)
    wd4_d = din("wd4", (128, NP * 32), BF16)
    bdec_d = din("bdec", (128, 96), FP32)
    out_d = nc.dram_tensor("out", [UC, B], FP32, kind="ExternalOutput").ap()

    RELU = True

    with tile.TileContext(nc) as tc:
        with (
            tc.tile_pool(name="const", bufs=1) as const,
            tc.tile_pool(name="h1p", bufs=4) as h1p,
            tc.tile_pool(name="h2p", bufs=4) as h2p,
            tc.tile_pool(name="h3p", bufs=4) as h3p,
            tc.tile_pool(name="stg", bufs=2) as stgp,
            tc.tile_pool(name="ps", bufs=3, space="PSUM") as psp,
            tc.tile_pool(name="pl4", bufs=1, space="PSUM") as pl4p,
        ):
            drain = _Drain(nc)

            def load(dst_shape, dtype, src, tag):
                t = const.tile(list(dst_shape), dtype, tag=tag, name=tag)
                nc.sync.dma_start(out=t[:], in_=src)
                return t

            # PE warm-up burst on memset data: ramps the PE p-state to
            # 2.4 GHz while the input DMAs land.  No DMA dependencies.
            wu = const.tile([128, 512], BF16, tag="wu", name="wu")
            nc.gpsimd.memset(wu[:], 0.0)
            wu_ps = psp.tile([128, CP], FP32, tag="ps", name="wu_ps")
            for i in range(10):
                nc.tensor.matmul(wu_ps[:, 0:CH], wu[:, 0:128], wu[:, 0:CH])

            # encoder weights + x chunk 0 first so the encoder starts early
            wenc = load((128, 512), BF16, wenc_d[:], "wenc")
            benc = load((128, 4), FP32, benc_d[:], "benc")
            we1 = wenc[:, 0:KT * H]
            we2 = wenc[0:H, 256:320]
            we3 = wenc[0:H, 320:384]
            we4 = wenc[0:H, 384:448]
            be1 = benc[0:H, 0:1]
            be2 = benc[0:H, 1:2]
            be3 = benc[0:H, 2:3]
            be4 = benc[0:H, 3:4]

            xts = [const.tile([128, B], BF16, tag=f"xt{t}", name=f"xt{t}")
                   for t in range(KT)]
            for t in range(KT):
                nc.sync.dma_start(out=xts[t][:, 0:CH], in_=xt_d[t][:, 0:CH])

            wd1 = load((64, NP * 128), BF16, wd1_d[:], "wd1")
            bdec = load((128, 96), FP32, bdec_d[:], "bdec")
            for c in range(1, NCHUNK):
                c0 = c * CH
                for t in range(KT):
                    nc.sync.dma_start(out=xts[t][:, c0:c0 + CH],
                                      in_=xt_d[t][:, c0:c0 + CH])
            wd2 = load((128, NP * 128), BF16, wd2_d[:], "wd2")
            wd3 = load((128, NP * 128), BF16, wd3_d[:], "wd3")
            wd4 = load((128, NP * 32), BF16, wd4_d[:], "wd4")
            bd1 = bdec[:, 0:32]
            bd2 = bdec[:, 32:64]
            bd3 = bdec[:, 64:96]

            z1 = const.tile([H, B], BF16, tag="z1", name="z1")
            z2 = const.tile([H, B], BF16, tag="z2", name="z2")
            z3 = const.tile([H, B], BF16, tag="z3", name="z3")
            zz = const.tile([H, B], BF16, tag="zz", name="zz")  # 2 z copies

            # ---------------- encoder (replicated), per 1024-cp ----------
            def enc_l1(cp):
                ps = psp.tile([128, CP], FP32, tag="ps", name=f"pe1_{cp}")
                for cc in range(2):
                    c0 = cp * CP + cc * CH
                    for t in range(KT):
                        nc.tensor.matmul(ps[0:H, cc * CH:(cc + 1) * CH],
                                         we1[:, t * H:(t + 1) * H],
                                         xts[t][:, c0:c0 + CH],
                                         start=(t == 0), stop=(t == KT - 1))
                sl = slice(cp * CP, (cp + 1) * CP)
                drain(z1[:, sl], ps[0:H, :], be1, RELU)

            def enc_mid(cp, win, bin_, zin, zout):
                ps = psp.tile([128, CP], FP32, tag="ps", name=f"pem_{cp}")
                for cc in range(2):
                    c0 = cp * CP + cc * CH
                    nc.tensor.matmul(ps[0:H, cc * CH:(cc + 1) * CH], win,
                                     zin[:, c0:c0 + CH])
                sl = slice(cp * CP, (cp + 1) * CP)
                drain(zout[:, sl], ps[0:H, :], bin_, RELU)

            def enc_l4(cp):
                ps = psp.tile([128, CP], FP32, tag="ps", name=f"pe4_{cp}")
                for cc in range(2):
                    c0 = cp * CP + cc * CH
                    nc.tensor.matmul(ps[0:H, cc * CH:(cc + 1) * CH], we4,
                                     z3[:, c0:c0 + CH])
                sl = slice(cp * CP, (cp + 1) * CP)
                drain(zz[:, sl], ps[0:H, :], be4, False)

            # diagonal pipeline over (layer, cp)
            for s in range(NCP + 3):
                for lyr in range(4):
                    cp = s - lyr
                    if 0 <= cp < NCP:
                        if lyr == 0:
                            enc_l1(cp)
                        elif lyr == 1:
                            enc_mid(cp, we2, be2, z1, z2)
                        elif lyr == 2:
                            enc_mid(cp, we3, be3, z2, z3)
                        else:
                            enc_l4(cp)

            # ---------------- decoder: 1-slot software pipeline ------------
            # slot j = cp*NP + p ; stages L1(j) | L2(j-1) | L3(j-2) | L4(j-3)
            T1 = [None] * NSLOT
            T2 = [None] * NSLOT
            T3 = [None] * NSLOT
            pl4_tiles = {}

            def stage_mm2(lhsT, rhs_tile, rhs_off, kpart, bias, Tout, name):
                """Two [*,512] matmuls into one 2-bank PSUM tile + one
                balanced drain into Tout."""
                ps = psp.tile([128, CP], FP32, tag="ps", name=name)
                for cc in range(2):
                    nc.tensor.matmul(
                        ps[:, cc * CH:(cc + 1) * CH], lhsT,
                        rhs_tile[0:kpart, rhs_off + cc * CH:
                                 rhs_off + (cc + 1) * CH],
                        tile_position=(0, 0))
                drain(Tout[:, :], ps[:, :], bias, RELU)

            def l1_stage(j):
                cp, p = divmod(j, NP)
                T1[j] = h1p.tile([128, CP], BF16, tag="t1", name=f"t1_{j}")
                stage_mm2(wd1[:, p * 128:(p + 1) * 128],
                          zz, cp * CP, H, bd1[:, p:p + 1], T1[j], f"l1_{j}")

            def l2_stage(j):
                cp, p = divmod(j, NP)
                T2[j] = h2p.tile([128, CP], BF16, tag="t2", name=f"t2_{j}")
                stage_mm2(wd2[:, p * 128:(p + 1) * 128],
                          T1[j], 0, 128, bd2[:, p:p + 1], T2[j], f"l2_{j}")
                T1[j] = None

            def l3_stage(j):
                cp, p = divmod(j, NP)
                T3[j] = h3p.tile([128, CP], BF16, tag="t3", name=f"t3_{j}")
                stage_mm2(wd3[:, p * 128:(p + 1) * 128],
                          T2[j], 0, 128, bd3[:, p:p + 1], T3[j], f"l3_{j}")
                T2[j] = None

            def l4_stage(j):
                cp, p = divmod(j, NP)
                g, jj = divmod(p, 4)
                key = (cp, g)
                if jj == 0:
                    pl4_tiles[key] = pl4p.tile([128, CP], FP32, tag="pl4",
                                               name=f"pl4_{cp}_{g}")
                pt = pl4_tiles[key]
                for cc in range(2):
                    nc.tensor.matmul(
                        pt[32 * jj:32 * jj + 32, cc * CH:(cc + 1) * CH],
                        wd4[:, 32 * p:32 * p + 32],
                        T3[j][:, cc * CH:(cc + 1) * CH],
                        tile_position=(0, 32 * jj))
                T3[j] = None
                if jj == 3:
                    sl = slice(cp * CP, (cp + 1) * CP)
                    stg = stgp.tile([128, CP], FP32, tag="stg",
                                    name=f"stg_{cp}_{g}")
                    drain(stg[:, :], pt[:, :], None, False)
                    for k in range(4):
                        nc.sync.dma_start(
                            out=out_d[8 * g + 2 * k:8 * g + 2 * k + 2, sl],
                            in_=stg[32 * k:32 * k + 2, :])
                    del pl4_tiles[key]

            S = 2
            for t in range(NSLOT + 3 * S):
                if t < NSLOT:
                    l1_stage(t)
                if 0 <= t - S < NSLOT:
                    l2_stage(t - S)
                if 0 <= t - 2 * S < NSLOT:
                    l3_stage(t - 2 * S)
                if 0 <= t - 3 * S < NSLOT:
                    l4_stage(t - 3 * S)

    nc.compile()
    return nc


def _get_program():
    global _PROG
    if _PROG is None:
        _PROG = _build_program()
    return _PROG


def kernel(x, We1, be1, We2, be2, We3, be3, We4, be4,
           Wd1, bd1, Wd2, bd2, Wd3, bd3, Wd4, bd4):
    global LAST_EXEC_NS, LAST_RESULTS
    shared = _pack_shared(np.asarray(x, np.float32),
                          np.asarray(We1, np.float32), np.asarray(be1, np.float32),
                          np.asarray(We2, np.float32), np.asarray(be2, np.float32),
                          np.asarray(We3, np.float32), np.asarray(be3, np.float32),
                          np.asarray(We4, np.float32), np.asarray(be4, np.float32))
    in_maps = []
    for c in range(NCORES):
        m = dict(shared)
        m.update(_pack_core(c, np.asarray(Wd1, np.float32), np.asarray(bd1, np.float32),
                            np.asarray(Wd2, np.float32), np.asarray(bd2, np.float32),
                            np.asarray(Wd3, np.float32), np.asarray(bd3, np.float32),
                            np.asarray(Wd4, np.float32)))
        in_maps.append(m)

    nc = _get_program()
    trace = bool(int(os.environ.get("BASSK_TRACE", "0")))
    kwargs = {}
    if trace:
        kwargs["tmpdir"] = os.environ.get("BASSK_TMPDIR") or None
    res = run_bass_kernel_spmd(nc, in_maps, core_ids=list(range(NCORES)),
                               trace=trace, **kwargs)
    LAST_EXEC_NS = res.exec_time_ns
    LAST_RESULTS = res

    outT = np.concatenate([res.results[c]["out"] for c in range(NCORES)], axis=0)
    out = outT.T.astype(np.float32) + np.asarray(bd4, np.float32)[None, :]
    return out
